# revision 2
# baseline (speedup 1.0000x reference)
"""Chebyshev approximation kernel for Trainium2 (8 NeuronCores, SPMD data-parallel).

Math: reference computes
    y_at_nodes = (1-t) * y[:, idx] + t * y[:, idx+1]      # [n_obs, deg]
    out        = (y_at_nodes @ basis).reshape(-1)         # [n_obs*deg]
Factorized device kernel (contraction 1024 instead of 2049):
    g0 = y[:, idx]   (GPSIMD ap_gather, runtime idx as data)
    g1 = y[:, idx+1]
    yn = g0 + t*(g1 - g0)          # DVE, written bf16
    out = yn @ basis               # PE: bf16 stationary = PE-transposed yn
Everything x-dependent (idx wrapped int16 lists, t replicated) is runtime
DATA, so one compile covers any x. bf16 GEMM + bf16 output store keep the
L2 rel err ~2e-3 (tolerance 2e-2) while halving PE work vs the y@C form
and cutting output DMA in half.

Sharding: y rows split 8192/core across 8 cores; basis/idx/t replicated.
"""

import os
import numpy as np

DEG = 1024
N_OBS = 65536
M_P1 = 2049
N_CORES = 8
ROWS_PER_CORE = N_OBS // N_CORES  # 8192
RB = 128                          # rows per block
KJ = 8                            # node k-tiles of 128 (1024/128)

_COMPILED = {}
_PREP_CACHE = {}
LAST_RESULTS = None


def _prep(x: np.ndarray):
    """Host precompute of idx/t/basis from x (cheap, O(deg^2))."""
    import ml_dtypes

    key = x.tobytes()
    hit = _PREP_CACHE.get(key)
    if hit is not None:
        return hit
    x = np.asarray(x, dtype=np.float32)
    k = np.arange(DEG, dtype=np.float32)
    ang = (np.float32(np.pi) * (k + np.float32(0.5))) / np.float32(DEG)
    nodes = np.sort(np.cos(ang.astype(np.float32)).astype(np.float32))
    idx = np.clip(np.searchsorted(x, nodes, side="right") - 1, 0, M_P1 - 2)
    a = x[idx]
    b = x[idx + 1]
    t = ((nodes - a) / (b - a)).astype(np.float32)          # [1024]
    norm = ((np.float32(2.0) - (k == 0).astype(np.float32)) / np.float32(DEG)).astype(
        np.float64
    )
    theta = np.arccos(nodes.astype(np.float64))
    basis = norm[None, :] * np.cos(k.astype(np.float64)[None, :] * theta[:, None])
    B_bf = np.ascontiguousarray(basis.astype(ml_dtypes.bfloat16))  # [1024 j, 1024 d]

    # ap_gather index layout: idx list wrapped mod 16 across partitions,
    # replicated into each 16-partition group (all 8 gpsimd cores).
    def wrap(ix):
        w = np.empty((128, DEG // 16), dtype=np.int16)
        p = np.arange(128) % 16
        s = np.arange(DEG // 16)
        w[:, :] = ix[(s[None, :] * 16 + p[:, None])]
        return np.ascontiguousarray(w)

    ix0 = wrap(idx.astype(np.int16))
    ix1 = wrap((idx + 1).astype(np.int16))
    w1rep = np.ascontiguousarray(np.tile(t[None, :], (128, 1)))  # [128, 1024] f32
    out = (B_bf, w1rep, ix0, ix1)
    _PREP_CACHE[key] = out
    return out


def build_cheb_kernel(tc, y_ap, b_ap, w1_ap, ix0_ap, ix1_ap, id_ap, o_ap, rows):
    """Per-core program: o[rows, DEG] (bf16) = lerp-gather(y) @ basis."""
    import concourse.mybir as mybir
    from concourse import library_config

    nc = tc.nc
    f32 = mybir.dt.float32
    bf16 = mybir.dt.bfloat16
    i16 = mybir.dt.int16
    nb = rows // RB
    G = 4  # transposes per PSUM group; 2 groups of 4 cover KJ=8

    nc.gpsimd.load_library(library_config.ap_gather)

    with (
        tc.tile_pool(name="consts", bufs=1) as consts,
        tc.tile_pool(name="ypool", bufs=4) as ypool,
        tc.tile_pool(name="g0pool", bufs=3) as g0pool,
        tc.tile_pool(name="g1pool", bufs=3) as g1pool,
        tc.tile_pool(name="dpool", bufs=2) as dpool,
        tc.tile_pool(name="qpool", bufs=2) as qpool,
        tc.tile_pool(name="ynpool", bufs=3) as ynpool,
        tc.tile_pool(name="ytpool", bufs=2) as ytpool,
        tc.tile_pool(name="opool", bufs=2) as opool,
        tc.tile_pool(name="pst", bufs=2, space="PSUM") as pstp,
        tc.tile_pool(name="pso", bufs=2, space="PSUM") as psop,
    ):
        ident = consts.tile([128, 128], bf16)
        nc.scalar.dma_start(out=ident, in_=id_ap)
        ix0 = consts.tile([128, DEG // 16], i16)
        nc.scalar.dma_start(out=ix0, in_=ix0_ap)
        ix1 = consts.tile([128, DEG // 16], i16)
        nc.scalar.dma_start(out=ix1, in_=ix1_ap)
        w1r = consts.tile([128, DEG], f32)
        nc.scalar.dma_start(out=w1r, in_=w1_ap)
        # basis resident in SBUF: [j-within-tile, jtile, d]; chunked DMAs
        # alternating queues so early y loads aren't starved.
        b_sb = consts.tile([128, KJ, DEG], bf16)
        b_r = b_ap.rearrange("(t p) n -> p t n", p=128)
        for kk in range(KJ):
            eng = nc.scalar if kk % 2 == 0 else nc.sync
            eng.dma_start(out=b_sb[:, kk, :], in_=b_r[:, kk, :])

        ybs, g0s, g1s, yns, ynTs, psos = {}, {}, {}, {}, {}, {}

        def load_y(b):
            yb = ypool.tile([128, M_P1], f32, name="yb", tag="yb")
            nc.sync.dma_start(out=yb, in_=y_ap[b * RB : (b + 1) * RB, :])
            ybs[b] = yb

        def gathers(b):
            g0 = g0pool.tile([128, DEG], f32, name="g0", tag="g0")
            nc.gpsimd.ap_gather(
                g0, ybs[b], ix0, channels=128, num_elems=M_P1, d=1, num_idxs=DEG
            )
            g1 = g1pool.tile([128, DEG], f32, name="g1", tag="g1")
            nc.gpsimd.ap_gather(
                g1, ybs[b], ix1, channels=128, num_elems=M_P1, d=1, num_idxs=DEG
            )
            g0s[b], g1s[b] = g0, g1
            del ybs[b]

        def lerp(b):
            g0, g1 = g0s[b], g1s[b]
            d = dpool.tile([128, DEG], f32, name="d", tag="d")
            nc.vector.tensor_sub(d, g1, g0)
            q = qpool.tile([128, DEG], f32, name="q", tag="q")
            nc.vector.tensor_mul(q, d, w1r)
            yn = ynpool.tile([128, DEG], bf16, name="yn", tag="yn")
            nc.vector.tensor_add(yn, q, g0)
            yns[b] = yn
            del g0s[b], g1s[b]

        def tgroup(b, g):
            if g == 0:
                ynTs[b] = ytpool.tile([128, KJ, 128], bf16, name="ynT", tag="ynT")
            pst = pstp.tile([128, G, 128], bf16, name="pst", tag="pst")
            yn = yns[b]
            for j in range(G):
                kk = g * G + j
                nc.tensor.transpose(
                    pst[:, j, :], yn[:, kk * 128 : (kk + 1) * 128], ident
                )
            dst = ynTs[b][:, g * G : (g + 1) * G, :]
            if g == 0:
                nc.vector.tensor_copy(dst, pst)
            else:
                nc.scalar.copy(dst, pst)
                del yns[b]

        def mgroup(b, g):
            if g == 0:
                psos[b] = psop.tile([128, DEG], mybir.dt.float32, name="ps", tag="ps")
            ps = psos[b]
            for j in range(G):
                kk = g * G + j
                for nh in range(2):
                    nc.tensor.matmul(
                        ps[:, nh * 512 : (nh + 1) * 512],
                        ynTs[b][:, kk, :],
                        b_sb[:, kk, nh * 512 : (nh + 1) * 512],
                        start=(kk == 0),
                        stop=(kk == KJ - 1),
                    )

        def store(b):
            osb = opool.tile([128, DEG], bf16, name="osb", tag="osb")
            nc.scalar.copy(osb, psos[b])
            nc.scalar.dma_start(out=o_ap[b * RB : (b + 1) * RB, :], in_=osb)
            del ynTs[b], psos[b]

        # prologue
        for b in range(min(3, nb)):
            load_y(b)
        gathers(0)
        if nb > 1:
            gathers(1)
        lerp(0)

        stages = [(b, g) for b in range(nb) for g in range(2)]
        for i, (b, g) in enumerate(stages):
            if g == 0:
                if b + 3 < nb:
                    load_y(b + 3)
                if b + 2 < nb:
                    gathers(b + 2)
            else:
                if b + 1 < nb:
                    lerp(b + 1)
            tgroup(b, g)
            if i >= 1:
                pb, pg = stages[i - 1]
                mgroup(pb, pg)
                if pg == 1:
                    store(pb)
        mgroup(nb - 1, 1)
        store(nb - 1)


def _build_nc(rows):
    import concourse.mybir as mybir
    import concourse.tile as tile
    from concourse import bacc

    f32 = mybir.dt.float32
    bf16 = mybir.dt.bfloat16
    i16 = mybir.dt.int16
    nc = bacc.Bacc(
        "TRN2",
        target_bir_lowering=False,
        debug=False,
        enable_asserts=False,
        num_devices=N_CORES,
    )
    y_ap = nc.dram_tensor("y", [rows, M_P1], f32, kind="ExternalInput").ap()
    b_ap = nc.dram_tensor("bmat", [DEG, DEG], bf16, kind="ExternalInput").ap()
    w1_ap = nc.dram_tensor("w1", [128, DEG], f32, kind="ExternalInput").ap()
    ix0_ap = nc.dram_tensor("ix0", [128, DEG // 16], i16, kind="ExternalInput").ap()
    ix1_ap = nc.dram_tensor("ix1", [128, DEG // 16], i16, kind="ExternalInput").ap()
    id_ap = nc.dram_tensor("ident", [128, 128], bf16, kind="ExternalInput").ap()
    o_ap = nc.dram_tensor("o", [rows, DEG], bf16, kind="ExternalOutput").ap()
    with tile.TileContext(nc) as tc:
        build_cheb_kernel(tc, y_ap, b_ap, w1_ap, ix0_ap, ix1_ap, id_ap, o_ap, rows)
    nc.compile()
    return nc


def _get_compiled(rows=ROWS_PER_CORE):
    if rows not in _COMPILED:
        _COMPILED[rows] = _build_nc(rows)
    return _COMPILED[rows]


def kernel(x: np.ndarray, y: np.ndarray) -> np.ndarray:
    global LAST_RESULTS
    import ml_dtypes
    from concourse import bass_utils

    x = np.asarray(x, dtype=np.float32)
    y = np.ascontiguousarray(np.asarray(y, dtype=np.float32))
    assert y.shape == (N_OBS, M_P1), y.shape
    B_bf, w1rep, ix0, ix1 = _prep(x)

    nc = _get_compiled()
    ident = np.ascontiguousarray(np.eye(128, dtype=ml_dtypes.bfloat16))
    in_maps = [
        {
            "y": y[i * ROWS_PER_CORE : (i + 1) * ROWS_PER_CORE],
            "bmat": B_bf,
            "w1": w1rep,
            "ix0": ix0,
            "ix1": ix1,
            "ident": ident,
        }
        for i in range(N_CORES)
    ]
    trace = bool(int(os.environ.get("CHEB_TRACE", "0")))
    res = bass_utils.run_bass_kernel_spmd(
        nc, in_maps, core_ids=list(range(N_CORES)), trace=trace
    )
    LAST_RESULTS = res
    out = np.concatenate(
        [
            np.asarray(res.results[i]["o"]).astype(np.float32)
            for i in range(N_CORES)
        ],
        axis=0,
    )
    return out.reshape(-1)


# revision 10
# speedup vs baseline: 7.1402x; 7.1402x over previous
"""Chebyshev approximation kernel for Trainium2 (8 NeuronCores, SPMD data-parallel).

Math: reference computes
    y_at_nodes = (1-t) * y[:, idx] + t * y[:, idx+1]      # [n_obs, deg]
    out        = (y_at_nodes @ basis).reshape(-1)         # [n_obs*deg]
Factorized device kernel: out = (y @ W) @ B where W [2049, 1024] holds the
two interp weights per node column and B is the dense basis. W is banded
(idx is monotone), so GEMM1 (ynT = W^T @ y^T) only touches ~26 of the 136
possible [128x128] tile pairs; its output lands in PSUM already transposed
for GEMM2 (contraction 1024 vs 2049 of the fused y@C form). All matmuls in
bf16 (full PE rate, LDWEIGHTS hidden); y is cast bf16 on DVE/ACT before the
PE transposes; GEMM1 runs on m=512 groups (4 row-blocks) to amortize weight
loads. Output stored bf16 (halves store DMA), upcast on host.

Sharding: y rows split 8192/core across 8 cores; W/B replicated. The band
structure (not the W values) is baked at compile time and cached by its
signature, so recompiles only happen if x changes shape qualitatively.
"""

import os
import numpy as np

DEG = 1024
N_OBS = 65536
M_P1 = 2049
N_CORES = 8
ROWS_PER_CORE = N_OBS // N_CORES  # 8192
RB = 128                          # rows per block
GB = 4                            # blocks per GEMM1 group (m = 512)
KT = 17                           # k tiles of 128 covering 2049 (pad to 2176)
KP = KT * 128                     # 2176
JT = 8                            # node j-tiles (1024/128)

_COMPILED = {}
_PREP_CACHE = {}
LAST_RESULTS = None


def _prep(x: np.ndarray):
    """Host precompute: banded W (bf16), basis B (bf16), band structure."""
    import ml_dtypes

    key = x.tobytes()
    hit = _PREP_CACHE.get(key)
    if hit is not None:
        return hit
    x = np.asarray(x, dtype=np.float32)
    k = np.arange(DEG, dtype=np.float32)
    ang = (np.float32(np.pi) * (k + np.float32(0.5))) / np.float32(DEG)
    nodes = np.sort(np.cos(ang.astype(np.float32)).astype(np.float32))
    idx = np.clip(np.searchsorted(x, nodes, side="right") - 1, 0, M_P1 - 2)
    a = x[idx]
    b = x[idx + 1]
    t = ((nodes - a) / (b - a)).astype(np.float64)
    W = np.zeros((KP, DEG), dtype=np.float64)
    W[idx, np.arange(DEG)] += 1.0 - t
    W[idx + 1, np.arange(DEG)] += t
    W_bf = np.ascontiguousarray(W.astype(ml_dtypes.bfloat16))

    norm = ((np.float32(2.0) - (k == 0).astype(np.float32)) / np.float32(DEG)).astype(
        np.float64
    )
    theta = np.arccos(nodes.astype(np.float64))
    basis = norm[None, :] * np.cos(k.astype(np.float64)[None, :] * theta[:, None])
    B_bf = np.ascontiguousarray(basis.astype(ml_dtypes.bfloat16))  # [1024 j, 1024 d]

    # band: per j-tile, the k-tiles containing any nonzero of W
    bands = []
    for jt in range(JT):
        lo = int(idx[jt * 128 : (jt + 1) * 128].min()) // 128
        hi = int(idx[jt * 128 : (jt + 1) * 128].max() + 1) // 128
        bands.append(tuple(range(lo, hi + 1)))
    bands = tuple(bands)
    out = (W_bf, B_bf, bands)
    _PREP_CACHE[key] = out
    return out


def build_cheb_kernel(tc, y_ap, w_ap, b_ap, id_ap, o_ap, rows, bands):
    import concourse.mybir as mybir

    nc = tc.nc
    f32 = mybir.dt.float32
    bf16 = mybir.dt.bfloat16
    nb = rows // RB
    ngrp = nb // GB

    with (
        tc.tile_pool(name="consts", bufs=1) as consts,
        tc.tile_pool(name="ypool", bufs=6) as ypool,
        tc.tile_pool(name="ycpool", bufs=6) as ycpool,
        tc.tile_pool(name="ytg", bufs=2) as ytgpool,
        tc.tile_pool(name="ynt", bufs=2) as yntpool,
        tc.tile_pool(name="opool", bufs=3) as opool,
        tc.tile_pool(name="pst", bufs=2, space="PSUM") as pstp,
        tc.tile_pool(name="p1", bufs=2, space="PSUM") as p1p,
        tc.tile_pool(name="pso", bufs=2, space="PSUM") as psop,
    ):
        ident = consts.tile([128, 128], bf16)
        nc.scalar.dma_start(out=ident, in_=id_ap)
        # B resident: [j-within-tile, jt, d]
        b_sb = consts.tile([128, JT, DEG], bf16)
        b_r = b_ap.rearrange("(t p) n -> p t n", p=128)
        for kk in range(JT):
            eng = nc.scalar if kk % 2 == 0 else nc.sync
            eng.dma_start(out=b_sb[:, kk, :], in_=b_r[:, kk, :])
        # W band tiles resident: [k-within-tile, band-slot, j]
        nband = sum(len(bd) for bd in bands)
        w_sb = consts.tile([128, nband, 128], bf16)
        w_r = w_ap.rearrange("(t p) j -> p t j", p=128)
        slot = {}
        s = 0
        for jt, bd in enumerate(bands):
            for kt in bd:
                eng = nc.scalar if s % 2 == 0 else nc.sync
                eng.dma_start(
                    out=w_sb[:, s, :], in_=w_r[:, kt, jt * 128 : (jt + 1) * 128]
                )
                slot[(jt, kt)] = s
                s += 1

        ybs, ycs, ytgs, ynts, psos = {}, {}, {}, {}, {}

        def load_y(b):
            yb = ypool.tile([128, M_P1], f32, name="yb", tag="yb")
            nc.sync.dma_start(out=yb, in_=y_ap[b * RB : (b + 1) * RB, :])
            ybs[b] = yb

        def cast_block(b):
            yc = ycpool.tile([128, KP], bf16, name="yc", tag="yc")
            nc.vector.memset(yc[:, M_P1:KP], 0.0)
            nc.scalar.copy(yc[:, 0:1024], ybs[b][:, 0:1024])
            nc.vector.tensor_copy(yc[:, 1024:M_P1], ybs[b][:, 1024:M_P1])
            ycs[b] = yc
            del ybs[b]

        def trans_block(b):
            g = b % GB
            if g == 0:
                ytgs[b // GB] = ytgpool.tile(
                    [128, KT, GB * 128], bf16, name="ytg", tag="ytg"
                )
            ytg = ytgs[b // GB]
            yc = ycs[b]
            pst = None
            for gg in range(5):  # transpose groups: 4,4,4,4,1
                kts = list(range(gg * 4, min(gg * 4 + 4, KT)))
                # one bank-aligned pst tile serves two groups (subtile-tracked)
                if gg % 2 == 0:
                    pst = pstp.tile([128, 8, 128], bf16, name="pst", tag="pst")
                base = (gg % 2) * 4
                for ji, kt in enumerate(kts):
                    nc.tensor.transpose(
                        pst[:, base + ji, :], yc[:, kt * 128 : (kt + 1) * 128], ident
                    )
                dst = ytg[:, kts[0] : kts[-1] + 1, g * 128 : (g + 1) * 128]
                src = pst[:, base : base + len(kts), :]
                if gg % 2 == 0:
                    nc.vector.tensor_copy(dst, src)
                else:
                    nc.scalar.copy(dst, src)
            del ycs[b]

        def gemm1(grp):
            ytg = ytgs[grp]
            ynt = yntpool.tile([128, JT, GB * 128], bf16, name="ynt", tag="ynt")
            ynts[grp] = ynt
            for jt in range(JT):
                bd = bands[jt]
                p1 = p1p.tile([128, GB * 128], f32, name="p1", tag="p1")
                for i, kt in enumerate(bd):
                    nc.tensor.matmul(
                        p1,
                        w_sb[:, slot[(jt, kt)], :],
                        ytg[:, kt, :],
                        start=(i == 0),
                        stop=(i == len(bd) - 1),
                    )
                if jt % 2 == 0:
                    nc.vector.tensor_copy(ynt[:, jt, :], p1)
                else:
                    nc.scalar.copy(ynt[:, jt, :], p1)
            del ytgs[grp]

        def gemm2(b):
            g = b % GB
            ynt = ynts[b // GB]
            ps = psop.tile([128, DEG], f32, name="ps", tag="ps")
            for jt in range(JT):
                for nh in range(2):
                    nc.tensor.matmul(
                        ps[:, nh * 512 : (nh + 1) * 512],
                        ynt[:, jt, g * 128 : (g + 1) * 128],
                        b_sb[:, jt, nh * 512 : (nh + 1) * 512],
                        start=(jt == 0),
                        stop=(jt == JT - 1),
                    )
            osb = opool.tile([128, DEG], bf16, name="osb", tag="osb")
            nc.scalar.copy(osb, ps)
            nc.scalar.dma_start(out=o_ap[b * RB : (b + 1) * RB, :], in_=osb)
            if g == GB - 1:
                del ynts[b // GB]

        # prologue: loads for groups 0 and 1; casts for group 0
        for b in range(min(2 * GB, nb)):
            load_y(b)
        for b in range(min(GB, nb)):
            cast_block(b)

        for grp in range(ngrp):
            for b in range((grp + 2) * GB, min((grp + 3) * GB, nb)):
                load_y(b)
            for b in range(grp * GB, (grp + 1) * GB):
                trans_block(b)
            gemm1(grp)
            for b in range((grp + 1) * GB, min((grp + 2) * GB, nb)):
                cast_block(b)
            for b in range(grp * GB, (grp + 1) * GB):
                gemm2(b)


def _build_nc(rows, bands):
    import concourse.mybir as mybir
    import concourse.tile as tile
    from concourse import bacc

    f32 = mybir.dt.float32
    bf16 = mybir.dt.bfloat16
    nc = bacc.Bacc(
        "TRN2",
        target_bir_lowering=False,
        debug=False,
        enable_asserts=False,
        num_devices=N_CORES,
    )
    y_ap = nc.dram_tensor("y", [rows, M_P1], f32, kind="ExternalInput").ap()
    w_ap = nc.dram_tensor("wmat", [KP, DEG], bf16, kind="ExternalInput").ap()
    b_ap = nc.dram_tensor("bmat", [DEG, DEG], bf16, kind="ExternalInput").ap()
    id_ap = nc.dram_tensor("ident", [128, 128], bf16, kind="ExternalInput").ap()
    o_ap = nc.dram_tensor("o", [rows, DEG], bf16, kind="ExternalOutput").ap()
    with tile.TileContext(nc) as tc:
        build_cheb_kernel(tc, y_ap, w_ap, b_ap, id_ap, o_ap, rows, bands)
    nc.compile()
    return nc


def _get_compiled(rows, bands):
    key = (rows, bands)
    if key not in _COMPILED:
        _COMPILED[key] = _build_nc(rows, bands)
    return _COMPILED[key]


def kernel(x: np.ndarray, y: np.ndarray) -> np.ndarray:
    global LAST_RESULTS
    import ml_dtypes
    from concourse import bass_utils

    x = np.asarray(x, dtype=np.float32)
    y = np.ascontiguousarray(np.asarray(y, dtype=np.float32))
    assert y.shape == (N_OBS, M_P1), y.shape
    W_bf, B_bf, bands = _prep(x)

    nc = _get_compiled(ROWS_PER_CORE, bands)
    ident = np.ascontiguousarray(np.eye(128, dtype=ml_dtypes.bfloat16))
    in_maps = [
        {
            "y": y[i * ROWS_PER_CORE : (i + 1) * ROWS_PER_CORE],
            "wmat": W_bf,
            "bmat": B_bf,
            "ident": ident,
        }
        for i in range(N_CORES)
    ]
    trace = bool(int(os.environ.get("CHEB_TRACE", "0")))
    res = bass_utils.run_bass_kernel_spmd(
        nc, in_maps, core_ids=list(range(N_CORES)), trace=trace
    )
    LAST_RESULTS = res
    out = np.concatenate(
        [
            np.asarray(res.results[i]["o"]).astype(np.float32)
            for i in range(N_CORES)
        ],
        axis=0,
    )
    return out.reshape(-1)


# revision 12
# speedup vs baseline: 7.8441x; 1.0986x over previous
"""Chebyshev approximation kernel for Trainium2 (8 NeuronCores, SPMD data-parallel).

Math: reference computes
    y_at_nodes = (1-t) * y[:, idx] + t * y[:, idx+1]      # [n_obs, deg]
    out        = (y_at_nodes @ basis).reshape(-1)         # [n_obs*deg]
Factorized device kernel: out = (y @ W) @ B where W [2049, 1024] holds the
two interp weights per node column and B is the dense basis. W is banded
(idx is monotone), so GEMM1 (ynT = W^T @ y^T) only touches ~26 of the 136
possible [128x128] tile pairs; its output lands in PSUM already transposed
for GEMM2 (contraction 1024 vs 2049 of the fused y@C form). All matmuls in
bf16 (full PE rate, LDWEIGHTS hidden); y is cast bf16 on DVE/ACT before the
PE transposes; GEMM1 runs on m=512 groups (4 row-blocks) to amortize weight
loads. Output stored bf16 (halves store DMA), upcast on host.

Sharding: y rows split 8192/core across 8 cores; W/B replicated. The band
structure (not the W values) is baked at compile time and cached by its
signature, so recompiles only happen if x changes shape qualitatively.
"""

import os
import numpy as np

DEG = 1024
N_OBS = 65536
M_P1 = 2049
N_CORES = 8
ROWS_PER_CORE = N_OBS // N_CORES  # 8192
RB = 128                          # rows per block
GB = 4                            # blocks per GEMM1 group (m = 512)
KT = 17                           # k tiles of 128 covering 2049 (pad to 2176)
KP = KT * 128                     # 2176
JT = 8                            # node j-tiles (1024/128)

_COMPILED = {}
_PREP_CACHE = {}
LAST_RESULTS = None


def _prep(x: np.ndarray):
    """Host precompute: banded W (bf16), basis B (bf16), band structure."""
    import ml_dtypes

    key = x.tobytes()
    hit = _PREP_CACHE.get(key)
    if hit is not None:
        return hit
    x = np.asarray(x, dtype=np.float32)
    k = np.arange(DEG, dtype=np.float32)
    ang = (np.float32(np.pi) * (k + np.float32(0.5))) / np.float32(DEG)
    nodes = np.sort(np.cos(ang.astype(np.float32)).astype(np.float32))
    idx = np.clip(np.searchsorted(x, nodes, side="right") - 1, 0, M_P1 - 2)
    a = x[idx]
    b = x[idx + 1]
    t = ((nodes - a) / (b - a)).astype(np.float64)
    W = np.zeros((KP, DEG), dtype=np.float64)
    W[idx, np.arange(DEG)] += 1.0 - t
    W[idx + 1, np.arange(DEG)] += t
    W_bf = np.ascontiguousarray(W.astype(ml_dtypes.bfloat16))

    norm = ((np.float32(2.0) - (k == 0).astype(np.float32)) / np.float32(DEG)).astype(
        np.float64
    )
    theta = np.arccos(nodes.astype(np.float64))
    basis = norm[None, :] * np.cos(k.astype(np.float64)[None, :] * theta[:, None])
    B_bf = np.ascontiguousarray(basis.astype(ml_dtypes.bfloat16))  # [1024 j, 1024 d]

    # band: per j-tile, the k-tiles containing any nonzero of W
    bands = []
    for jt in range(JT):
        lo = int(idx[jt * 128 : (jt + 1) * 128].min()) // 128
        hi = int(idx[jt * 128 : (jt + 1) * 128].max() + 1) // 128
        bands.append(tuple(range(lo, hi + 1)))
    bands = tuple(bands)
    out = (W_bf, B_bf, bands)
    _PREP_CACHE[key] = out
    return out


def build_cheb_kernel(tc, y_ap, w_ap, b_ap, id_ap, o_ap, rows, bands):
    import concourse.mybir as mybir

    nc = tc.nc
    f32 = mybir.dt.float32
    bf16 = mybir.dt.bfloat16
    nb = rows // RB
    ngrp = nb // GB

    with (
        tc.tile_pool(name="consts", bufs=1) as consts,
        tc.tile_pool(name="ypool", bufs=6) as ypool,
        tc.tile_pool(name="ycpool", bufs=6) as ycpool,
        tc.tile_pool(name="ytg", bufs=2) as ytgpool,
        tc.tile_pool(name="ynt", bufs=2) as yntpool,
        tc.tile_pool(name="opool", bufs=3) as opool,
        tc.tile_pool(name="pst", bufs=3, space="PSUM") as pstp,
        tc.tile_pool(name="p1", bufs=2, space="PSUM") as p1p,
        tc.tile_pool(name="pso", bufs=3, space="PSUM") as psop,
    ):
        ident = consts.tile([128, 128], bf16)
        nc.scalar.dma_start(out=ident, in_=id_ap)
        # B resident: [j-within-tile, jt, d]
        b_sb = consts.tile([128, JT, DEG], bf16)
        b_r = b_ap.rearrange("(t p) n -> p t n", p=128)
        for kk in range(JT):
            eng = nc.scalar if kk % 2 == 0 else nc.sync
            eng.dma_start(out=b_sb[:, kk, :], in_=b_r[:, kk, :])
        # W band tiles resident: [k-within-tile, band-slot, j]
        nband = sum(len(bd) for bd in bands)
        w_sb = consts.tile([128, nband, 128], bf16)
        w_r = w_ap.rearrange("(t p) j -> p t j", p=128)
        slot = {}
        s = 0
        for jt, bd in enumerate(bands):
            for kt in bd:
                eng = nc.scalar if s % 2 == 0 else nc.sync
                eng.dma_start(
                    out=w_sb[:, s, :], in_=w_r[:, kt, jt * 128 : (jt + 1) * 128]
                )
                slot[(jt, kt)] = s
                s += 1

        ybs, ycs, ytgs, ynts, psos = {}, {}, {}, {}, {}

        def load_y(b):
            yb = ypool.tile([128, M_P1], f32, name="yb", tag="yb")
            nc.sync.dma_start(out=yb, in_=y_ap[b * RB : (b + 1) * RB, :])
            ybs[b] = yb

        def cast_block(b):
            yc = ycpool.tile([128, KP], bf16, name="yc", tag="yc")
            nc.vector.memset(yc[:, M_P1:KP], 0.0)
            nc.scalar.copy(yc[:, 0:1024], ybs[b][:, 0:1024])
            nc.vector.tensor_copy(yc[:, 1024:M_P1], ybs[b][:, 1024:M_P1])
            ycs[b] = yc
            del ybs[b]

        def trans_block(b):
            g = b % GB
            if g == 0:
                ytgs[b // GB] = ytgpool.tile(
                    [128, KT, GB * 128], bf16, name="ytg", tag="ytg"
                )
            ytg = ytgs[b // GB]
            yc = ycs[b]
            pst = None
            for gg in range(5):  # transpose groups: 4,4,4,4,1
                kts = list(range(gg * 4, min(gg * 4 + 4, KT)))
                # one bank-aligned pst tile serves two groups (subtile-tracked)
                if gg % 2 == 0:
                    pst = pstp.tile([128, 8, 128], bf16, name="pst", tag="pst")
                base = (gg % 2) * 4
                for ji, kt in enumerate(kts):
                    nc.tensor.transpose(
                        pst[:, base + ji, :], yc[:, kt * 128 : (kt + 1) * 128], ident
                    )
                dst = ytg[:, kts[0] : kts[-1] + 1, g * 128 : (g + 1) * 128]
                src = pst[:, base : base + len(kts), :]
                if gg % 2 == 0:
                    nc.vector.tensor_copy(dst, src)
                else:
                    nc.scalar.copy(dst, src)
            del ycs[b]

        def gemm1(grp):
            ytg = ytgs[grp]
            ynt = yntpool.tile([128, JT, GB * 128], bf16, name="ynt", tag="ynt")
            ynts[grp] = ynt
            for jt in range(JT):
                bd = bands[jt]
                p1 = p1p.tile([128, GB * 128], f32, name="p1", tag="p1")
                for i, kt in enumerate(bd):
                    nc.tensor.matmul(
                        p1,
                        w_sb[:, slot[(jt, kt)], :],
                        ytg[:, kt, :],
                        start=(i == 0),
                        stop=(i == len(bd) - 1),
                    )
                if jt % 2 == 0:
                    nc.vector.tensor_copy(ynt[:, jt, :], p1)
                else:
                    nc.scalar.copy(ynt[:, jt, :], p1)
            del ytgs[grp]

        def gemm2(b):
            g = b % GB
            ynt = ynts[b // GB]
            osb = opool.tile([128, DEG], bf16, name="osb", tag="osb")
            for nh in range(2):
                ps = psop.tile([128, 512], f32, name="ps", tag="ps")
                for jt in range(JT):
                    nc.tensor.matmul(
                        ps,
                        ynt[:, jt, g * 128 : (g + 1) * 128],
                        b_sb[:, jt, nh * 512 : (nh + 1) * 512],
                        start=(jt == 0),
                        stop=(jt == JT - 1),
                    )
                nc.scalar.copy(osb[:, nh * 512 : (nh + 1) * 512], ps)
            nc.scalar.dma_start(out=o_ap[b * RB : (b + 1) * RB, :], in_=osb)
            if g == GB - 1:
                del ynts[b // GB]

        # prologue: loads for groups 0 and 1; casts for group 0
        for b in range(min(2 * GB, nb)):
            load_y(b)
        for b in range(min(GB, nb)):
            cast_block(b)

        for grp in range(ngrp):
            for b in range((grp + 2) * GB, min((grp + 3) * GB, nb)):
                load_y(b)
            for b in range(grp * GB, (grp + 1) * GB):
                trans_block(b)
            gemm1(grp)
            for b in range((grp + 1) * GB, min((grp + 2) * GB, nb)):
                cast_block(b)
            for b in range(grp * GB, (grp + 1) * GB):
                gemm2(b)


def _build_nc(rows, bands):
    import concourse.mybir as mybir
    import concourse.tile as tile
    from concourse import bacc

    f32 = mybir.dt.float32
    bf16 = mybir.dt.bfloat16
    nc = bacc.Bacc(
        "TRN2",
        target_bir_lowering=False,
        debug=False,
        enable_asserts=False,
        num_devices=N_CORES,
    )
    y_ap = nc.dram_tensor("y", [rows, M_P1], f32, kind="ExternalInput").ap()
    w_ap = nc.dram_tensor("wmat", [KP, DEG], bf16, kind="ExternalInput").ap()
    b_ap = nc.dram_tensor("bmat", [DEG, DEG], bf16, kind="ExternalInput").ap()
    id_ap = nc.dram_tensor("ident", [128, 128], bf16, kind="ExternalInput").ap()
    o_ap = nc.dram_tensor("o", [rows, DEG], bf16, kind="ExternalOutput").ap()
    with tile.TileContext(nc) as tc:
        build_cheb_kernel(tc, y_ap, w_ap, b_ap, id_ap, o_ap, rows, bands)
    nc.compile()
    return nc


def _get_compiled(rows, bands):
    key = (rows, bands)
    if key not in _COMPILED:
        _COMPILED[key] = _build_nc(rows, bands)
    return _COMPILED[key]


def kernel(x: np.ndarray, y: np.ndarray) -> np.ndarray:
    global LAST_RESULTS
    import ml_dtypes
    from concourse import bass_utils

    x = np.asarray(x, dtype=np.float32)
    y = np.ascontiguousarray(np.asarray(y, dtype=np.float32))
    assert y.shape == (N_OBS, M_P1), y.shape
    W_bf, B_bf, bands = _prep(x)

    nc = _get_compiled(ROWS_PER_CORE, bands)
    ident = np.ascontiguousarray(np.eye(128, dtype=ml_dtypes.bfloat16))
    in_maps = [
        {
            "y": y[i * ROWS_PER_CORE : (i + 1) * ROWS_PER_CORE],
            "wmat": W_bf,
            "bmat": B_bf,
            "ident": ident,
        }
        for i in range(N_CORES)
    ]
    trace = bool(int(os.environ.get("CHEB_TRACE", "0")))
    res = bass_utils.run_bass_kernel_spmd(
        nc, in_maps, core_ids=list(range(N_CORES)), trace=trace
    )
    LAST_RESULTS = res
    out = np.concatenate(
        [
            np.asarray(res.results[i]["o"]).astype(np.float32)
            for i in range(N_CORES)
        ],
        axis=0,
    )
    return out.reshape(-1)


# revision 14
# speedup vs baseline: 7.9498x; 1.0135x over previous
"""Chebyshev approximation kernel for Trainium2 (8 NeuronCores, SPMD data-parallel).

Math: reference computes
    y_at_nodes = (1-t) * y[:, idx] + t * y[:, idx+1]      # [n_obs, deg]
    out        = (y_at_nodes @ basis).reshape(-1)         # [n_obs*deg]
Factorized device kernel: out = (y @ W) @ B where W [2049, 1024] holds the
two interp weights per node column and B is the dense basis. W is banded
(idx is monotone), so GEMM1 (ynT = W^T @ y^T) only touches ~26 of the 136
possible [128x128] tile pairs; its output lands in PSUM already transposed
for GEMM2 (contraction 1024 vs 2049 of the fused y@C form). All matmuls in
bf16 (full PE rate, LDWEIGHTS hidden); y is cast bf16 on DVE/ACT before the
PE transposes; GEMM1 runs on m=512 groups (4 row-blocks) to amortize weight
loads. Output stored bf16 (halves store DMA), upcast on host.

Sharding: y rows split 8192/core across 8 cores; W/B replicated. The band
structure (not the W values) is baked at compile time and cached by its
signature, so recompiles only happen if x changes shape qualitatively.
"""

import os
import numpy as np

DEG = 1024
N_OBS = 65536
M_P1 = 2049
N_CORES = 8
ROWS_PER_CORE = N_OBS // N_CORES  # 8192
RB = 128                          # rows per block
GB = 4                            # blocks per GEMM1 group (m = 512)
KT = 17                           # k tiles of 128 covering 2049 (pad to 2176)
KP = KT * 128                     # 2176
JT = 8                            # node j-tiles (1024/128)

_COMPILED = {}
_PREP_CACHE = {}
LAST_RESULTS = None


def _prep(x: np.ndarray):
    """Host precompute: banded W (bf16), basis B (bf16), band structure."""
    import ml_dtypes

    key = x.tobytes()
    hit = _PREP_CACHE.get(key)
    if hit is not None:
        return hit
    x = np.asarray(x, dtype=np.float32)
    k = np.arange(DEG, dtype=np.float32)
    ang = (np.float32(np.pi) * (k + np.float32(0.5))) / np.float32(DEG)
    nodes = np.sort(np.cos(ang.astype(np.float32)).astype(np.float32))
    idx = np.clip(np.searchsorted(x, nodes, side="right") - 1, 0, M_P1 - 2)
    a = x[idx]
    b = x[idx + 1]
    t = ((nodes - a) / (b - a)).astype(np.float64)
    W = np.zeros((KP, DEG), dtype=np.float64)
    W[idx, np.arange(DEG)] += 1.0 - t
    W[idx + 1, np.arange(DEG)] += t
    W_bf = np.ascontiguousarray(W.astype(ml_dtypes.bfloat16))

    norm = ((np.float32(2.0) - (k == 0).astype(np.float32)) / np.float32(DEG)).astype(
        np.float64
    )
    theta = np.arccos(nodes.astype(np.float64))
    basis = norm[None, :] * np.cos(k.astype(np.float64)[None, :] * theta[:, None])
    B_bf = np.ascontiguousarray(basis.astype(ml_dtypes.bfloat16))  # [1024 j, 1024 d]

    # band: per j-tile, the k-tiles containing any nonzero of W
    bands = []
    for jt in range(JT):
        lo = int(idx[jt * 128 : (jt + 1) * 128].min()) // 128
        hi = int(idx[jt * 128 : (jt + 1) * 128].max() + 1) // 128
        bands.append(tuple(range(lo, hi + 1)))
    bands = tuple(bands)
    out = (W_bf, B_bf, bands)
    _PREP_CACHE[key] = out
    return out


def build_cheb_kernel(tc, y_ap, w_ap, b_ap, id_ap, o_ap, rows, bands):
    import concourse.mybir as mybir

    nc = tc.nc
    f32 = mybir.dt.float32
    bf16 = mybir.dt.bfloat16
    nb = rows // RB
    ngrp = nb // GB

    with (
        tc.tile_pool(name="consts", bufs=1) as consts,
        tc.tile_pool(name="ypool", bufs=6) as ypool,
        tc.tile_pool(name="ycpool", bufs=6) as ycpool,
        tc.tile_pool(name="ytg", bufs=2) as ytgpool,
        tc.tile_pool(name="ynt", bufs=2) as yntpool,
        tc.tile_pool(name="opool", bufs=3) as opool,
        tc.tile_pool(name="pst", bufs=3, space="PSUM") as pstp,
        tc.tile_pool(name="p1", bufs=2, space="PSUM") as p1p,
        tc.tile_pool(name="pso", bufs=3, space="PSUM") as psop,
    ):
        ident = consts.tile([128, 128], bf16)
        nc.scalar.dma_start(out=ident, in_=id_ap)
        b_sb = consts.tile([128, JT, DEG], bf16)
        nband = sum(len(bd) for bd in bands)
        w_sb = consts.tile([128, nband, 128], bf16)

        def load_consts():
            # Emitted after the first y-block loads so the pipeline head
            # isn't serialized behind ~2.7MB of constants.
            b_r = b_ap.rearrange("(t p) n -> p t n", p=128)
            for kk in range(JT):
                eng = nc.scalar if kk % 2 == 0 else nc.sync
                eng.dma_start(out=b_sb[:, kk, :], in_=b_r[:, kk, :])
            w_r = w_ap.rearrange("(t p) j -> p t j", p=128)
            s = 0
            for jt, bd in enumerate(bands):
                for kt in bd:
                    eng = nc.scalar if s % 2 == 0 else nc.sync
                    eng.dma_start(
                        out=w_sb[:, s, :], in_=w_r[:, kt, jt * 128 : (jt + 1) * 128]
                    )
                    slot[(jt, kt)] = s
                    s += 1

        slot = {}
        ybs, ycs, ytgs, ynts, psos = {}, {}, {}, {}, {}

        def load_y(b):
            yb = ypool.tile([128, M_P1], f32, name="yb", tag="yb")
            nc.sync.dma_start(out=yb, in_=y_ap[b * RB : (b + 1) * RB, :])
            ybs[b] = yb

        def cast_block(b):
            yc = ycpool.tile([128, KP], bf16, name="yc", tag="yc")
            nc.vector.memset(yc[:, M_P1:KP], 0.0)
            nc.scalar.copy(yc[:, 0:1024], ybs[b][:, 0:1024])
            nc.vector.tensor_copy(yc[:, 1024:M_P1], ybs[b][:, 1024:M_P1])
            ycs[b] = yc
            del ybs[b]

        def trans_block(b):
            g = b % GB
            if g == 0:
                ytgs[b // GB] = ytgpool.tile(
                    [128, KT, GB * 128], bf16, name="ytg", tag="ytg"
                )
            ytg = ytgs[b // GB]
            yc = ycs[b]
            pst = None
            for gg in range(5):  # transpose groups: 4,4,4,4,1
                kts = list(range(gg * 4, min(gg * 4 + 4, KT)))
                # one bank-aligned pst tile serves two groups (subtile-tracked)
                if gg % 2 == 0:
                    pst = pstp.tile([128, 8, 128], bf16, name="pst", tag="pst")
                base = (gg % 2) * 4
                for ji, kt in enumerate(kts):
                    nc.tensor.transpose(
                        pst[:, base + ji, :], yc[:, kt * 128 : (kt + 1) * 128], ident
                    )
                dst = ytg[:, kts[0] : kts[-1] + 1, g * 128 : (g + 1) * 128]
                src = pst[:, base : base + len(kts), :]
                if gg % 2 == 0:
                    nc.vector.tensor_copy(dst, src)
                else:
                    nc.scalar.copy(dst, src)
            del ycs[b]

        def gemm1(grp):
            ytg = ytgs[grp]
            ynt = yntpool.tile([128, JT, GB * 128], bf16, name="ynt", tag="ynt")
            ynts[grp] = ynt
            for jt in range(JT):
                bd = bands[jt]
                p1 = p1p.tile([128, GB * 128], f32, name="p1", tag="p1")
                for i, kt in enumerate(bd):
                    nc.tensor.matmul(
                        p1,
                        w_sb[:, slot[(jt, kt)], :],
                        ytg[:, kt, :],
                        start=(i == 0),
                        stop=(i == len(bd) - 1),
                    )
                if jt % 2 == 0:
                    nc.vector.tensor_copy(ynt[:, jt, :], p1)
                else:
                    nc.scalar.copy(ynt[:, jt, :], p1)
            del ytgs[grp]

        def gemm2(b):
            g = b % GB
            ynt = ynts[b // GB]
            osb = opool.tile([128, DEG], bf16, name="osb", tag="osb")
            for nh in range(2):
                ps = psop.tile([128, 512], f32, name="ps", tag="ps")
                for jt in range(JT):
                    nc.tensor.matmul(
                        ps,
                        ynt[:, jt, g * 128 : (g + 1) * 128],
                        b_sb[:, jt, nh * 512 : (nh + 1) * 512],
                        start=(jt == 0),
                        stop=(jt == JT - 1),
                    )
                nc.scalar.copy(osb[:, nh * 512 : (nh + 1) * 512], ps)
            nc.scalar.dma_start(out=o_ap[b * RB : (b + 1) * RB, :], in_=osb)
            if g == GB - 1:
                del ynts[b // GB]

        # prologue: first-group y loads beat the constant loads onto the
        # queues; W tiles land before gemm1(0), B before gemm2(0).
        for b in range(min(GB, nb)):
            load_y(b)
        load_consts()
        for b in range(GB, min(2 * GB, nb)):
            load_y(b)
        for b in range(min(GB, nb)):
            cast_block(b)

        for grp in range(ngrp):
            for b in range((grp + 2) * GB, min((grp + 3) * GB, nb)):
                load_y(b)
            for b in range(grp * GB, (grp + 1) * GB):
                trans_block(b)
            for b in range((grp + 1) * GB, min((grp + 2) * GB, nb)):
                cast_block(b)
            gemm1(grp)
            for b in range(grp * GB, (grp + 1) * GB):
                gemm2(b)


def _build_nc(rows, bands):
    import concourse.mybir as mybir
    import concourse.tile as tile
    from concourse import bacc

    f32 = mybir.dt.float32
    bf16 = mybir.dt.bfloat16
    nc = bacc.Bacc(
        "TRN2",
        target_bir_lowering=False,
        debug=False,
        enable_asserts=False,
        num_devices=N_CORES,
    )
    y_ap = nc.dram_tensor("y", [rows, M_P1], f32, kind="ExternalInput").ap()
    w_ap = nc.dram_tensor("wmat", [KP, DEG], bf16, kind="ExternalInput").ap()
    b_ap = nc.dram_tensor("bmat", [DEG, DEG], bf16, kind="ExternalInput").ap()
    id_ap = nc.dram_tensor("ident", [128, 128], bf16, kind="ExternalInput").ap()
    o_ap = nc.dram_tensor("o", [rows, DEG], bf16, kind="ExternalOutput").ap()
    with tile.TileContext(nc) as tc:
        build_cheb_kernel(tc, y_ap, w_ap, b_ap, id_ap, o_ap, rows, bands)
    nc.compile()
    return nc


def _get_compiled(rows, bands):
    key = (rows, bands)
    if key not in _COMPILED:
        _COMPILED[key] = _build_nc(rows, bands)
    return _COMPILED[key]


def kernel(x: np.ndarray, y: np.ndarray) -> np.ndarray:
    global LAST_RESULTS
    import ml_dtypes
    from concourse import bass_utils

    x = np.asarray(x, dtype=np.float32)
    y = np.ascontiguousarray(np.asarray(y, dtype=np.float32))
    assert y.shape == (N_OBS, M_P1), y.shape
    W_bf, B_bf, bands = _prep(x)

    nc = _get_compiled(ROWS_PER_CORE, bands)
    ident = np.ascontiguousarray(np.eye(128, dtype=ml_dtypes.bfloat16))
    in_maps = [
        {
            "y": y[i * ROWS_PER_CORE : (i + 1) * ROWS_PER_CORE],
            "wmat": W_bf,
            "bmat": B_bf,
            "ident": ident,
        }
        for i in range(N_CORES)
    ]
    trace = bool(int(os.environ.get("CHEB_TRACE", "0")))
    res = bass_utils.run_bass_kernel_spmd(
        nc, in_maps, core_ids=list(range(N_CORES)), trace=trace
    )
    LAST_RESULTS = res
    out = np.concatenate(
        [
            np.asarray(res.results[i]["o"]).astype(np.float32)
            for i in range(N_CORES)
        ],
        axis=0,
    )
    return out.reshape(-1)
